# revision 16
# baseline (speedup 1.0000x reference)
"""GCN edge-aggregation kernel for 8 Trainium2 NeuronCores.

Math (see nn_GCNEdge): h = relu((segment_sum(edge_data, dst) / max(count,1)) @ W.T + b)

Strategy
--------
Host-side (sharding/layout only):
  * Nodes are dealt into 784 blocks of <=128 nodes with a serpentine deal over
    degree-sorted nodes (plus a small swap-repair pass), equalizing per-block
    edge counts so every block fits K_CHUNKS*128 = 2048 edge slots.  8 cores
    x 98 blocks; the output is un-permuted on the host at the end.
  * Each edge is routed to the core/block owning its destination node; within
    a block, edges occupy sequential slots padded to K*128.
  * Edge features ship as single bf16 (the rel-err budget is 2e-2; bf16 with
    f32 PSUM accumulation lands ~5e-3).  Blocks are paired so each input DMA
    moves ~1 MiB with 8 KiB contiguous lines.
  * Per-node reciprocal mean weights (1/max(degree,1)) ship as a tiny
    [128, 98] f32 tensor — degree counts already exist host-side from the
    balancing pass, so no count column and no phantom edges are needed.

Device-side (per core, per 128-node block):
  * One-hot of local node ids via one DVE is_equal in [p, n*K+c] layout: the
    lid operand broadcasts along n with a stride-1 innermost c axis, so all
    operands are packed 2-byte APs and the op runs in 2x DVE mode (the naive
    [p, c*128+n] layout has a stride-0 innermost broadcast -> 1x mode).
  * PE matmul-accumulate onehot.T @ x into PSUM (chunk c of the one-hot is
    the strided slice [:, c, :]) -> per-node feature sums,
  * mean = sums * rec[node] via one ACT copy-scale directly from PSUM,
  * PE transpose, then out = relu(W @ agg.T + b) in bf16 (fp32 PE matmuls run
    at 1/4 rate), output stays [out_feat, node] bf16 and is un-transposed,
    un-permuted, and cast to f32 on the host.
  * Output DMAs issue on the ACT HWDGE ring so they never head-of-line block
    the input-pair DMAs on the SP ring.

No collectives: output shards are disjoint.
"""

import numpy as np
import ml_dtypes

BF16 = ml_dtypes.bfloat16

N_NODES = 100000
N_EDGES = 1600000
F = 128
N_CORES = 8
BLK = 128                       # nodes per block
BLOCKS_PER_CORE = 98
TOTAL_BLOCKS = N_CORES * BLOCKS_PER_CORE        # 784
NODES_PER_CORE = BLOCKS_PER_CORE * BLK          # 12544
K_CHUNKS = 16                   # 128-edge chunks per block (capacity 2048)

_module_cache = {}


def _build_module(K):
    import concourse.mybir as mybir
    import concourse.tile as tile
    from concourse import bacc

    f32 = mybir.dt.float32
    bf16 = mybir.dt.bfloat16
    B = BLOCKS_PER_CORE
    NPAIR = B // 2
    W128 = K * 128               # per-block row width

    nc = bacc.Bacc("TRN2", target_bir_lowering=False, debug=False)
    # xe rows are (pair, partition); each row holds both blocks' K chunks of
    # 128 bf16 values contiguously -> ~1 MiB DMAs with 8 KiB-contiguous lines.
    xe = nc.dram_tensor("xe", [NPAIR * 128, 2 * W128], bf16, kind="ExternalInput")
    lid = nc.dram_tensor("lid", [128, B * K], bf16, kind="ExternalInput")
    # iotar[p, n*K + c] = n
    iotar = nc.dram_tensor("iotar", [128, 128 * K], bf16, kind="ExternalInput")
    recs = nc.dram_tensor("recs", [128, B], f32, kind="ExternalInput")
    wt = nc.dram_tensor("wt", [128, 128], bf16, kind="ExternalInput")
    bias = nc.dram_tensor("bias", [128, 1], f32, kind="ExternalInput")
    ident = nc.dram_tensor("ident", [128, 128], bf16, kind="ExternalInput")
    out = nc.dram_tensor("out", [128, B * 128], bf16, kind="ExternalOutput")

    xe_ap = xe.ap()
    out_ap = out.ap()

    with tile.TileContext(nc) as tc:
        with (
            tc.tile_pool(name="const", bufs=1) as cpool,
            tc.tile_pool(name="xp", bufs=4) as xpool,
            tc.tile_pool(name="ohp", bufs=8) as ohpool,
            tc.tile_pool(name="ep", bufs=3) as epool,
            tc.tile_pool(name="psS", bufs=4, space="PSUM") as psS,
            tc.tile_pool(name="psT", bufs=2, space="PSUM") as psT,
            tc.tile_pool(name="psO", bufs=2, space="PSUM") as psO,
        ):
            # iotar/lid gate the very first one-hot: put them FIRST on the
            # SP ring, ahead of the input pairs.  On the ACT ring they
            # compete with the pair flood at packet granularity and finish
            # ~20us in, leaving PE idle-then-cold and stalling the stream
            # when the xt buffers fill.  The epilogue-only constants ride
            # the ACT ring.
            iotar_t = cpool.tile([128, 128 * K], bf16)
            nc.sync.dma_start(iotar_t[:], iotar.ap()[:])
            lid_t = cpool.tile([128, B * K], bf16)
            nc.sync.dma_start(lid_t[:], lid.ap()[:])
            wt_t = cpool.tile([128, 128], bf16)
            nc.scalar.dma_start(wt_t[:], wt.ap()[:])
            bias_t = cpool.tile([128, 1], f32)
            nc.scalar.dma_start(bias_t[:], bias.ap()[:])
            id_t = cpool.tile([128, 128], bf16)
            nc.scalar.dma_start(id_t[:], ident.ap()[:])
            rec_t = cpool.tile([128, B], f32)
            nc.scalar.dma_start(rec_t[:], recs.ap()[:])

            group_pT = {}

            def emit_oh(b):
                oh = ohpool.tile([128, 128 * K], bf16, name=f"oh{b}", tag="oh")
                nc.vector.tensor_tensor(
                    out=oh[:].rearrange("p (n c) -> p n c", c=K),
                    in0=iotar_t[:].rearrange("p (n c) -> p n c", c=K),
                    in1=lid_t[:, b * K:(b + 1) * K].unsqueeze(1).to_broadcast(
                        [128, 128, K]
                    ),
                    op=mybir.AluOpType.is_equal,
                )
                return oh

            def emit_matmuls(b, xt, oh):
                ps = psS.tile([128, 128], f32, name=f"ps{b}", tag="ps")
                half = (b % 2) * W128
                ohc = oh[:].rearrange("p (n c) -> p c n", c=K)
                for c in range(K):
                    nc.tensor.matmul(
                        ps[:],
                        lhsT=ohc[:, c, :],
                        rhs=xt[:, half + c * 128:half + (c + 1) * 128],
                        start=(c == 0),
                        stop=(c == K - 1),
                    )
                return ps

            def emit_epilogue(b, ps):
                # agg = sums * (1/max(deg,1)) straight from PSUM; rec is a
                # preloaded per-node constant, so the only cross-engine dep is
                # the PE accumulation stop.  agg/pT are bf16: the transpose
                # runs 1 cycle/row instead of fp32's 2.
                agg = epool.tile([128, 128], bf16, name=f"agg{b}", tag="agg")
                nc.scalar.activation(
                    agg[:], ps[:],
                    mybir.ActivationFunctionType.Copy, scale=rec_t[:, b:b + 1],
                )
                j = b % 4
                if j == 0:
                    group_pT["t"] = psT.tile([128, 512], bf16, name=f"pT{b}", tag="pT")
                pT = group_pT["t"]
                nc.tensor.transpose(pT[:, j * 128:(j + 1) * 128], agg[:], id_t[:])
                if j == 3 or b == B - 1:
                    g0 = (b // 4) * 4
                    gw = (b + 1 - g0) * 128
                    aggT = epool.tile([128, 512], bf16, name=f"aggT{b}", tag="aggT", bufs=2)
                    nc.scalar.copy(aggT[:, 0:gw], pT[:, 0:gw])
                    pO = psO.tile([128, 512], f32, name=f"pO{b}", tag="pO")
                    nc.tensor.matmul(
                        pO[:, 0:gw], lhsT=wt_t[:], rhs=aggT[:, 0:gw],
                        start=True, stop=True,
                    )
                    ot = epool.tile([128, 512], bf16, name=f"ot{b}", tag="ot", bufs=2)
                    nc.scalar.activation(
                        ot[:, 0:gw], pO[:, 0:gw],
                        mybir.ActivationFunctionType.Relu,
                        bias=bias_t[:, 0:1], scale=1.0,
                    )
                    # ACT HWDGE ring: output DMAs wait on the deep epilogue
                    # pipeline and would head-of-line block the input-pair
                    # DMAs on the SP ring.
                    nc.scalar.dma_start(out_ap[:, g0 * 128:(b + 1) * 128], ot[:, 0:gw])

            # Software-pipelined emission; per-engine queues are strict
            # in-order, so each stage's dependencies are blocks old by the
            # time its queue reaches the op:
            #   iter b:  DMA pair(b/2) | one-hot(b) | PE matmuls(b-1)
            #            | epilogue(b-3)
            pending_ps = {}
            xt_of = {}
            for b in range(B):
                if b % 2 == 0:
                    q = b // 2
                    xt = xpool.tile([128, 2 * W128], bf16, name=f"xt{q}", tag="xt")
                    nc.sync.dma_start(xt[:], xe_ap[q * 128:(q + 1) * 128, :])
                    xt_of[b] = xt
                    xt_of[b + 1] = xt
                oh = emit_oh(b)
                pending_ps[b] = emit_matmuls(b, xt_of.pop(b), oh)
                if b >= 3:
                    emit_epilogue(b - 3, pending_ps.pop(b - 3))
            for bb in sorted(pending_ps):
                emit_epilogue(bb, pending_ps.pop(bb))

    nc.compile()
    return nc


def _get_module(K):
    if K not in _module_cache:
        _module_cache[K] = _build_module(K)
    return _module_cache[K]


def _balance_nodes(deg):
    """Assign nodes to TOTAL_BLOCKS bins of <=128 nodes, equalizing per-bin
    edge load.  Returns (node_bin, node_loc, K)."""
    w = np.maximum(deg, 1).astype(np.int64)
    order = np.argsort(-w, kind="stable")
    S = TOTAL_BLOCKS * BLK
    idx = np.arange(S)
    r = idx // TOTAL_BLOCKS
    k = idx % TOTAL_BLOCKS
    bins = np.where(r % 2 == 0, k, TOTAL_BLOCKS - 1 - k)   # serpentine deal
    node_bin = np.empty(N_NODES, np.int64)
    node_loc = np.empty(N_NODES, np.int64)
    node_bin[order] = bins[:N_NODES]
    node_loc[order] = r[:N_NODES]
    loads = np.zeros(TOTAL_BLOCKS, np.int64)
    np.add.at(loads, node_bin, deg)

    cap = K_CHUNKS * BLK
    for _ in range(2000):
        hot = int(loads.argmax())
        if loads[hot] <= cap:
            break
        cold = int(loads.argmin())
        need = int(loads[hot] - cap)
        headroom = int(cap - loads[cold])
        hot_nodes = np.nonzero(node_bin == hot)[0]
        cold_nodes = np.nonzero(node_bin == cold)[0]
        diffs = deg[hot_nodes][:, None] - deg[cold_nodes][None, :]
        mask = (diffs >= 1) & (diffs <= headroom)
        if not mask.any():
            break
        cand = np.where(mask, np.abs(diffs - need), 1 << 30)
        ai, bi = np.unravel_index(np.argmin(cand), cand.shape)
        na, nb = hot_nodes[ai], cold_nodes[bi]
        node_bin[na], node_bin[nb] = node_bin[nb], node_bin[na]
        node_loc[na], node_loc[nb] = node_loc[nb], node_loc[na]
        d = int(deg[na] - deg[nb])
        loads[hot] -= d
        loads[cold] += d

    K = max(K_CHUNKS, int(np.ceil(loads.max() / BLK)))
    return node_bin, node_loc, K


def prepare_inputs(edge_data, dst, W, b):
    """Host-side sharding: route each edge to the core/block owning dst."""
    edge_data = np.asarray(edge_data, dtype=np.float32)
    dst = np.asarray(dst).astype(np.int64)
    W = np.asarray(W, dtype=np.float32)
    b = np.asarray(b, dtype=np.float32)
    E = dst.shape[0]

    deg = np.bincount(dst, minlength=N_NODES)[:N_NODES]
    node_bin, node_loc, K = _balance_nodes(deg)
    RB = K * 128
    TOT = TOTAL_BLOCKS * RB

    eblk = node_bin[dst]
    cnt = np.bincount(eblk, minlength=TOTAL_BLOCKS)
    starts = np.zeros(TOTAL_BLOCKS, np.int64)
    np.cumsum(cnt[:-1], out=starts[1:])
    eorder = np.argsort(eblk, kind="stable")
    rank = np.empty(E, np.int64)
    rank[eorder] = np.arange(E, dtype=np.int64) - np.repeat(starts, cnt)
    slot = eblk * RB + rank

    X = np.zeros((TOT, 128), BF16)
    X[slot] = edge_data.astype(BF16)
    # [block, chunk, part, 128] -> [pair, part, 2 * chunk * 128]: each SBUF
    # partition's data is one 8 KiB contiguous HBM run, ~1 MiB per DMA.
    Xb = X.reshape(TOTAL_BLOCKS, K, 128, 128).transpose(0, 2, 1, 3)
    Xb = Xb.reshape(N_CORES, BLOCKS_PER_CORE // 2, 2, 128, K * 128)
    xe_all = np.ascontiguousarray(Xb.transpose(0, 1, 3, 2, 4)).reshape(
        N_CORES, (BLOCKS_PER_CORE // 2) * 128, 2 * K * 128
    )

    lid_f = np.full(TOT, -1.0, np.float32)
    lid_f[slot] = node_loc[dst].astype(np.float32)
    lid_all = (
        lid_f.reshape(N_CORES, BLOCKS_PER_CORE, K, 128)
        .transpose(0, 3, 1, 2)
        .reshape(N_CORES, 128, BLOCKS_PER_CORE * K)
        .astype(BF16)
    )
    rec_full = np.ones(TOTAL_BLOCKS * BLK, np.float32)
    rec_full[node_bin * BLK + node_loc] = 1.0 / np.maximum(deg, 1)
    rec_all = (
        rec_full.reshape(N_CORES, BLOCKS_PER_CORE, BLK).transpose(0, 2, 1)
    )                                            # [core, 128, B]
    wt = np.ascontiguousarray(W.T).astype(BF16)
    bias = np.ascontiguousarray(b.reshape(128, 1))
    ident = np.eye(128, dtype=np.float32).astype(BF16)
    iotar = np.ascontiguousarray(
        np.broadcast_to(
            np.repeat(np.arange(128, dtype=np.float32), K), (128, 128 * K)
        )
    ).astype(BF16)

    in_maps = [
        {
            "xe": np.ascontiguousarray(xe_all[c]),
            "lid": np.ascontiguousarray(lid_all[c]),
            "recs": np.ascontiguousarray(rec_all[c]),
            "wt": wt,
            "bias": bias,
            "ident": ident,
            "iotar": iotar,
        }
        for c in range(N_CORES)
    ]
    pos = node_bin * BLK + node_loc          # output column of each node
    return K, in_maps, pos


def run(edge_data, dst, W, b, trace=False, tmpdir=None):
    from concourse.bass_utils import run_bass_kernel_spmd

    K, in_maps, pos = prepare_inputs(edge_data, dst, W, b)
    nc = _get_module(K)
    res = run_bass_kernel_spmd(
        nc, in_maps, core_ids=list(range(N_CORES)), trace=trace, tmpdir=tmpdir,
    )
    full = np.concatenate(
        [res.results[c]["out"] for c in range(N_CORES)], axis=1
    )                                         # [128, 100352] bf16
    out = full.T[pos].astype(np.float32)      # un-permute -> [100000, 128]
    return np.ascontiguousarray(out), res


def kernel(edge_data, dst, W, b):
    out, _ = run(edge_data, dst, W, b, trace=False)
    return out


# revision 18
# speedup vs baseline: 1.1558x; 1.1558x over previous
"""GCN edge-aggregation kernel for 8 Trainium2 NeuronCores.

Math (see nn_GCNEdge): h = relu((segment_sum(edge_data, dst) / max(count,1)) @ W.T + b)

Strategy
--------
Host-side (sharding/layout only):
  * Nodes are dealt into 784 blocks of <=128 nodes with a serpentine deal over
    degree-sorted nodes (plus a small swap-repair pass), equalizing per-block
    edge counts so every block fits K_CHUNKS*128 = 2048 edge slots.  8 cores
    x 98 blocks; the output is un-permuted on the host at the end.
  * Each edge is routed to the core/block owning its destination node; within
    a block, edges occupy sequential slots padded to K*128.
  * Edge features ship as single bf16 (the rel-err budget is 2e-2; bf16 with
    f32 PSUM accumulation lands ~5e-3).  Blocks are paired so each input DMA
    moves ~1 MiB with 8 KiB contiguous lines.
  * Per-node reciprocal mean weights (1/max(degree,1)) ship as a tiny
    [128, 98] f32 tensor — degree counts already exist host-side from the
    balancing pass, so no count column and no phantom edges are needed.

Device-side (per core, per 128-node block):
  * One-hot of local node ids via one DVE is_equal in [p, n*K+c] layout: the
    lid operand broadcasts along n with a stride-1 innermost c axis, so all
    operands are packed 2-byte APs and the op runs in 2x DVE mode (the naive
    [p, c*128+n] layout has a stride-0 innermost broadcast -> 1x mode).
  * PE matmul-accumulate onehot.T @ x into PSUM (chunk c of the one-hot is
    the strided slice [:, c, :]) -> per-node feature sums,
  * mean = sums * rec[node] via one ACT copy-scale directly from PSUM,
  * PE transpose, then out = relu(W @ agg.T + b) in bf16 (fp32 PE matmuls run
    at 1/4 rate), output stays [out_feat, node] bf16 and is un-transposed,
    un-permuted, and cast to f32 on the host.
  * Output DMAs issue on the ACT HWDGE ring so they never head-of-line block
    the input-pair DMAs on the SP ring.

No collectives: output shards are disjoint.
"""

import numpy as np
import ml_dtypes

BF16 = ml_dtypes.bfloat16

N_NODES = 100000
N_EDGES = 1600000
F = 128
N_CORES = 8
BLK = 128                       # nodes per block
BLOCKS_PER_CORE = 98
TOTAL_BLOCKS = N_CORES * BLOCKS_PER_CORE        # 784
NODES_PER_CORE = BLOCKS_PER_CORE * BLK          # 12544
K_CHUNKS = 16                   # 128-edge chunks per block (capacity 2048)

_module_cache = {}


def _build_module(K):
    import concourse.mybir as mybir
    import concourse.tile as tile
    from concourse import bacc

    f32 = mybir.dt.float32
    bf16 = mybir.dt.bfloat16
    B = BLOCKS_PER_CORE
    NPAIR = B // 2
    W128 = K * 128               # per-block row width

    nc = bacc.Bacc("TRN2", target_bir_lowering=False, debug=False)
    # xe rows are (pair, partition); each row holds both blocks' K chunks of
    # 128 bf16 values contiguously -> ~1 MiB DMAs with 8 KiB-contiguous lines.
    xe = nc.dram_tensor("xe", [NPAIR * 128, 2 * W128], bf16, kind="ExternalInput")
    lid_h = nc.dram_tensor("lid_h", [128, 8 * K], bf16, kind="ExternalInput")
    lid_r = nc.dram_tensor("lid_r", [128, (B - 8) * K], bf16, kind="ExternalInput")
    # iotar[n*K + c] = n, one row; replicated across partitions on device
    iotar = nc.dram_tensor("iotar", [1, 128 * K], bf16, kind="ExternalInput")
    recs = nc.dram_tensor("recs", [128, B], f32, kind="ExternalInput")
    wt = nc.dram_tensor("wt", [128, 128], bf16, kind="ExternalInput")
    bias = nc.dram_tensor("bias", [128, 1], f32, kind="ExternalInput")
    ident = nc.dram_tensor("ident", [128, 128], bf16, kind="ExternalInput")
    out = nc.dram_tensor("out", [128, B * 128], bf16, kind="ExternalOutput")

    xe_ap = xe.ap()
    out_ap = out.ap()

    with tile.TileContext(nc) as tc:
        with (
            tc.tile_pool(name="const", bufs=1) as cpool,
            tc.tile_pool(name="xp", bufs=4) as xpool,
            tc.tile_pool(name="ohp", bufs=8) as ohpool,
            tc.tile_pool(name="ep", bufs=3) as epool,
            tc.tile_pool(name="psS", bufs=4, space="PSUM") as psS,
            tc.tile_pool(name="psT", bufs=2, space="PSUM") as psT,
            tc.tile_pool(name="psO", bufs=2, space="PSUM") as psO,
        ):
            # Constants ride the ACT HWDGE ring so the first input-pair DMA
            # on the SP ring starts immediately.  The one-hot inputs gate the
            # whole compute pipeline, so they are kept tiny: iotar ships as a
            # single 4KB row (replicated across partitions by the idle Pool
            # engine) and the first 8 blocks' lid slice arrives ahead of the
            # bulk so DVE starts within a few us instead of ~21us.
            iotar_row = cpool.tile([1, 128 * K], bf16)
            nc.scalar.dma_start(iotar_row[:], iotar.ap()[:])
            lid_ht = cpool.tile([128, 8 * K], bf16)
            nc.scalar.dma_start(lid_ht[:], lid_h.ap()[:])
            iotar_t = cpool.tile([128, 128 * K], bf16)
            nc.gpsimd.partition_broadcast(iotar_t[:], iotar_row[:])
            wt_t = cpool.tile([128, 128], bf16)
            nc.scalar.dma_start(wt_t[:], wt.ap()[:])
            bias_t = cpool.tile([128, 1], f32)
            nc.scalar.dma_start(bias_t[:], bias.ap()[:])
            id_t = cpool.tile([128, 128], bf16)
            nc.scalar.dma_start(id_t[:], ident.ap()[:])
            rec_t = cpool.tile([128, B], f32)
            nc.scalar.dma_start(rec_t[:], recs.ap()[:])
            lid_rt = cpool.tile([128, (B - 8) * K], bf16)
            nc.scalar.dma_start(lid_rt[:], lid_r.ap()[:])

            group_pT = {}

            def emit_oh(b):
                oh = ohpool.tile([128, 128 * K], bf16, name=f"oh{b}", tag="oh")
                nc.vector.tensor_tensor(
                    out=oh[:].rearrange("p (n c) -> p n c", c=K),
                    in0=iotar_t[:].rearrange("p (n c) -> p n c", c=K),
                    in1=(
                        lid_ht[:, b * K:(b + 1) * K]
                        if b < 8
                        else lid_rt[:, (b - 8) * K:(b - 7) * K]
                    ).unsqueeze(1).to_broadcast([128, 128, K]),
                    op=mybir.AluOpType.is_equal,
                )
                return oh

            def emit_matmuls(b, xt, oh):
                ps = psS.tile([128, 128], f32, name=f"ps{b}", tag="ps")
                half = (b % 2) * W128
                ohc = oh[:].rearrange("p (n c) -> p c n", c=K)
                for c in range(K):
                    nc.tensor.matmul(
                        ps[:],
                        lhsT=ohc[:, c, :],
                        rhs=xt[:, half + c * 128:half + (c + 1) * 128],
                        start=(c == 0),
                        stop=(c == K - 1),
                    )
                return ps

            def emit_epilogue(b, ps):
                # agg = sums * (1/max(deg,1)) straight from PSUM; rec is a
                # preloaded per-node constant, so the only cross-engine dep is
                # the PE accumulation stop.  agg/pT are bf16: the transpose
                # runs 1 cycle/row instead of fp32's 2.
                agg = epool.tile([128, 128], bf16, name=f"agg{b}", tag="agg")
                nc.scalar.activation(
                    agg[:], ps[:],
                    mybir.ActivationFunctionType.Copy, scale=rec_t[:, b:b + 1],
                )
                j = b % 4
                if j == 0:
                    group_pT["t"] = psT.tile([128, 512], bf16, name=f"pT{b}", tag="pT")
                pT = group_pT["t"]
                nc.tensor.transpose(pT[:, j * 128:(j + 1) * 128], agg[:], id_t[:])
                if j == 3 or b == B - 1:
                    g0 = (b // 4) * 4
                    gw = (b + 1 - g0) * 128
                    aggT = epool.tile([128, 512], bf16, name=f"aggT{b}", tag="aggT", bufs=2)
                    nc.scalar.copy(aggT[:, 0:gw], pT[:, 0:gw])
                    pO = psO.tile([128, 512], f32, name=f"pO{b}", tag="pO")
                    nc.tensor.matmul(
                        pO[:, 0:gw], lhsT=wt_t[:], rhs=aggT[:, 0:gw],
                        start=True, stop=True,
                    )
                    ot = epool.tile([128, 512], bf16, name=f"ot{b}", tag="ot", bufs=2)
                    nc.scalar.activation(
                        ot[:, 0:gw], pO[:, 0:gw],
                        mybir.ActivationFunctionType.Relu,
                        bias=bias_t[:, 0:1], scale=1.0,
                    )
                    # ACT HWDGE ring: output DMAs wait on the deep epilogue
                    # pipeline and would head-of-line block the input-pair
                    # DMAs on the SP ring.
                    nc.scalar.dma_start(out_ap[:, g0 * 128:(b + 1) * 128], ot[:, 0:gw])

            # Software-pipelined emission; per-engine queues are strict
            # in-order, so each stage's dependencies are blocks old by the
            # time its queue reaches the op:
            #   iter b:  DMA pair(b/2) | one-hot(b) | PE matmuls(b-1)
            #            | epilogue(b-3)
            pending_ps = {}
            xt_of = {}
            for b in range(B):
                if b % 2 == 0:
                    q = b // 2
                    xt = xpool.tile([128, 2 * W128], bf16, name=f"xt{q}", tag="xt")
                    nc.sync.dma_start(xt[:], xe_ap[q * 128:(q + 1) * 128, :])
                    xt_of[b] = xt
                    xt_of[b + 1] = xt
                oh = emit_oh(b)
                pending_ps[b] = emit_matmuls(b, xt_of.pop(b), oh)
                if b >= 3:
                    emit_epilogue(b - 3, pending_ps.pop(b - 3))
            for bb in sorted(pending_ps):
                emit_epilogue(bb, pending_ps.pop(bb))

    nc.compile()
    return nc


def _get_module(K):
    if K not in _module_cache:
        _module_cache[K] = _build_module(K)
    return _module_cache[K]


def _balance_nodes(deg):
    """Assign nodes to TOTAL_BLOCKS bins of <=128 nodes, equalizing per-bin
    edge load.  Returns (node_bin, node_loc, K)."""
    w = np.maximum(deg, 1).astype(np.int64)
    order = np.argsort(-w, kind="stable")
    S = TOTAL_BLOCKS * BLK
    idx = np.arange(S)
    r = idx // TOTAL_BLOCKS
    k = idx % TOTAL_BLOCKS
    bins = np.where(r % 2 == 0, k, TOTAL_BLOCKS - 1 - k)   # serpentine deal
    node_bin = np.empty(N_NODES, np.int64)
    node_loc = np.empty(N_NODES, np.int64)
    node_bin[order] = bins[:N_NODES]
    node_loc[order] = r[:N_NODES]
    loads = np.zeros(TOTAL_BLOCKS, np.int64)
    np.add.at(loads, node_bin, deg)

    cap = K_CHUNKS * BLK
    for _ in range(2000):
        hot = int(loads.argmax())
        if loads[hot] <= cap:
            break
        cold = int(loads.argmin())
        need = int(loads[hot] - cap)
        headroom = int(cap - loads[cold])
        hot_nodes = np.nonzero(node_bin == hot)[0]
        cold_nodes = np.nonzero(node_bin == cold)[0]
        diffs = deg[hot_nodes][:, None] - deg[cold_nodes][None, :]
        mask = (diffs >= 1) & (diffs <= headroom)
        if not mask.any():
            break
        cand = np.where(mask, np.abs(diffs - need), 1 << 30)
        ai, bi = np.unravel_index(np.argmin(cand), cand.shape)
        na, nb = hot_nodes[ai], cold_nodes[bi]
        node_bin[na], node_bin[nb] = node_bin[nb], node_bin[na]
        node_loc[na], node_loc[nb] = node_loc[nb], node_loc[na]
        d = int(deg[na] - deg[nb])
        loads[hot] -= d
        loads[cold] += d

    K = max(K_CHUNKS, int(np.ceil(loads.max() / BLK)))
    return node_bin, node_loc, K


def prepare_inputs(edge_data, dst, W, b):
    """Host-side sharding: route each edge to the core/block owning dst."""
    edge_data = np.asarray(edge_data, dtype=np.float32)
    dst = np.asarray(dst).astype(np.int64)
    W = np.asarray(W, dtype=np.float32)
    b = np.asarray(b, dtype=np.float32)
    E = dst.shape[0]

    deg = np.bincount(dst, minlength=N_NODES)[:N_NODES]
    node_bin, node_loc, K = _balance_nodes(deg)
    RB = K * 128
    TOT = TOTAL_BLOCKS * RB

    eblk = node_bin[dst]
    cnt = np.bincount(eblk, minlength=TOTAL_BLOCKS)
    starts = np.zeros(TOTAL_BLOCKS, np.int64)
    np.cumsum(cnt[:-1], out=starts[1:])
    eorder = np.argsort(eblk, kind="stable")
    rank = np.empty(E, np.int64)
    rank[eorder] = np.arange(E, dtype=np.int64) - np.repeat(starts, cnt)
    slot = eblk * RB + rank

    X = np.zeros((TOT, 128), BF16)
    X[slot] = edge_data.astype(BF16)
    # [block, chunk, part, 128] -> [pair, part, 2 * chunk * 128]: each SBUF
    # partition's data is one 8 KiB contiguous HBM run, ~1 MiB per DMA.
    Xb = X.reshape(TOTAL_BLOCKS, K, 128, 128).transpose(0, 2, 1, 3)
    Xb = Xb.reshape(N_CORES, BLOCKS_PER_CORE // 2, 2, 128, K * 128)
    xe_all = np.ascontiguousarray(Xb.transpose(0, 1, 3, 2, 4)).reshape(
        N_CORES, (BLOCKS_PER_CORE // 2) * 128, 2 * K * 128
    )

    lid_f = np.full(TOT, -1.0, np.float32)
    lid_f[slot] = node_loc[dst].astype(np.float32)
    lid_all = (
        lid_f.reshape(N_CORES, BLOCKS_PER_CORE, K, 128)
        .transpose(0, 3, 1, 2)
        .reshape(N_CORES, 128, BLOCKS_PER_CORE * K)
        .astype(BF16)
    )
    rec_full = np.ones(TOTAL_BLOCKS * BLK, np.float32)
    rec_full[node_bin * BLK + node_loc] = 1.0 / np.maximum(deg, 1)
    rec_all = (
        rec_full.reshape(N_CORES, BLOCKS_PER_CORE, BLK).transpose(0, 2, 1)
    )                                            # [core, 128, B]
    wt = np.ascontiguousarray(W.T).astype(BF16)
    bias = np.ascontiguousarray(b.reshape(128, 1))
    ident = np.eye(128, dtype=np.float32).astype(BF16)
    iotar = np.ascontiguousarray(
        np.repeat(np.arange(128, dtype=np.float32), K).reshape(1, 128 * K)
    ).astype(BF16)

    in_maps = [
        {
            "xe": np.ascontiguousarray(xe_all[c]),
            "lid_h": np.ascontiguousarray(lid_all[c][:, :8 * K]),
            "lid_r": np.ascontiguousarray(lid_all[c][:, 8 * K:]),
            "recs": np.ascontiguousarray(rec_all[c]),
            "wt": wt,
            "bias": bias,
            "ident": ident,
            "iotar": iotar,
        }
        for c in range(N_CORES)
    ]
    pos = node_bin * BLK + node_loc          # output column of each node
    return K, in_maps, pos


def run(edge_data, dst, W, b, trace=False, tmpdir=None):
    from concourse.bass_utils import run_bass_kernel_spmd

    K, in_maps, pos = prepare_inputs(edge_data, dst, W, b)
    nc = _get_module(K)
    res = run_bass_kernel_spmd(
        nc, in_maps, core_ids=list(range(N_CORES)), trace=trace, tmpdir=tmpdir,
    )
    full = np.concatenate(
        [res.results[c]["out"] for c in range(N_CORES)], axis=1
    )                                         # [128, 100352] bf16
    out = full.T[pos].astype(np.float32)      # un-permute -> [100000, 128]
    return np.ascontiguousarray(out), res


def kernel(edge_data, dst, W, b):
    out, _ = run(edge_data, dst, W, b, trace=False)
    return out


# revision 19
# speedup vs baseline: 1.1718x; 1.0138x over previous
"""GCN edge-aggregation kernel for 8 Trainium2 NeuronCores.

Math (see nn_GCNEdge): h = relu((segment_sum(edge_data, dst) / max(count,1)) @ W.T + b)

Strategy
--------
Host-side (sharding/layout only):
  * Nodes are dealt into 784 blocks of <=128 nodes with a serpentine deal over
    degree-sorted nodes (plus a small swap-repair pass), equalizing per-block
    edge counts so every block fits K_CHUNKS*128 = 2048 edge slots.  8 cores
    x 98 blocks; the output is un-permuted on the host at the end.
  * Each edge is routed to the core/block owning its destination node; within
    a block, edges occupy sequential slots padded to K*128.
  * Edge features ship as single bf16 (the rel-err budget is 2e-2; bf16 with
    f32 PSUM accumulation lands ~5e-3).  Blocks are paired so each input DMA
    moves ~1 MiB with 8 KiB contiguous lines.
  * Per-node reciprocal mean weights (1/max(degree,1)) ship as a tiny
    [128, 98] f32 tensor — degree counts already exist host-side from the
    balancing pass, so no count column and no phantom edges are needed.

Device-side (per core, per 128-node block):
  * One-hot of local node ids via one DVE is_equal in [p, n*K+c] layout: the
    lid operand broadcasts along n with a stride-1 innermost c axis, so all
    operands are packed 2-byte APs and the op runs in 2x DVE mode (the naive
    [p, c*128+n] layout has a stride-0 innermost broadcast -> 1x mode).
  * PE matmul-accumulate onehot.T @ x into PSUM (chunk c of the one-hot is
    the strided slice [:, c, :]) -> per-node feature sums,
  * mean = sums * rec[node] via one ACT copy-scale directly from PSUM,
  * PE transpose, then out = relu(W @ agg.T + b) in bf16 (fp32 PE matmuls run
    at 1/4 rate), output stays [out_feat, node] bf16 and is un-transposed,
    un-permuted, and cast to f32 on the host.
  * Output DMAs issue on the ACT HWDGE ring so they never head-of-line block
    the input-pair DMAs on the SP ring.

No collectives: output shards are disjoint.
"""

import numpy as np
import ml_dtypes

BF16 = ml_dtypes.bfloat16

N_NODES = 100000
N_EDGES = 1600000
F = 128
N_CORES = 8
BLK = 128                       # nodes per block
BLOCKS_PER_CORE = 98
TOTAL_BLOCKS = N_CORES * BLOCKS_PER_CORE        # 784
NODES_PER_CORE = BLOCKS_PER_CORE * BLK          # 12544
K_CHUNKS = 16                   # 128-edge chunks per block (capacity 2048)

_module_cache = {}


def _build_module(K):
    import concourse.mybir as mybir
    import concourse.tile as tile
    from concourse import bacc

    f32 = mybir.dt.float32
    bf16 = mybir.dt.bfloat16
    B = BLOCKS_PER_CORE
    NPAIR = B // 2
    W128 = K * 128               # per-block row width

    nc = bacc.Bacc("TRN2", target_bir_lowering=False, debug=False)
    # xe rows are (pair, partition); each row holds both blocks' K chunks of
    # 128 bf16 values contiguously -> ~1 MiB DMAs with 8 KiB-contiguous lines.
    xe = nc.dram_tensor("xe", [NPAIR * 128, 2 * W128], bf16, kind="ExternalInput")
    lid = nc.dram_tensor("lid", [128, B * K], bf16, kind="ExternalInput")
    # iotar[p, n*K + c] = n
    iotar = nc.dram_tensor("iotar", [128, 128 * K], bf16, kind="ExternalInput")
    recs = nc.dram_tensor("recs", [128, B], f32, kind="ExternalInput")
    wt = nc.dram_tensor("wt", [128, 128], bf16, kind="ExternalInput")
    bias = nc.dram_tensor("bias", [128, 1], f32, kind="ExternalInput")
    ident = nc.dram_tensor("ident", [128, 128], bf16, kind="ExternalInput")
    out = nc.dram_tensor("out", [128, B * 128], bf16, kind="ExternalOutput")

    xe_ap = xe.ap()
    out_ap = out.ap()

    with tile.TileContext(nc) as tc:
        with (
            tc.tile_pool(name="const", bufs=1) as cpool,
            tc.tile_pool(name="xp", bufs=4) as xpool,
            tc.tile_pool(name="ohp", bufs=8) as ohpool,
            tc.tile_pool(name="ep", bufs=3) as epool,
            tc.tile_pool(name="psS", bufs=4, space="PSUM") as psS,
            tc.tile_pool(name="psT", bufs=2, space="PSUM") as psT,
            tc.tile_pool(name="psO", bufs=2, space="PSUM") as psO,
        ):
            # Constants ride the ACT HWDGE ring so the first input-pair DMA
            # on the SP ring starts immediately.
            wt_t = cpool.tile([128, 128], bf16)
            nc.scalar.dma_start(wt_t[:], wt.ap()[:])
            bias_t = cpool.tile([128, 1], f32)
            nc.scalar.dma_start(bias_t[:], bias.ap()[:])
            id_t = cpool.tile([128, 128], bf16)
            nc.scalar.dma_start(id_t[:], ident.ap()[:])
            iotar_t = cpool.tile([128, 128 * K], bf16)
            nc.scalar.dma_start(iotar_t[:], iotar.ap()[:])
            lid_t = cpool.tile([128, B * K], bf16)
            nc.scalar.dma_start(lid_t[:], lid.ap()[:])
            rec_t = cpool.tile([128, B], f32)
            nc.scalar.dma_start(rec_t[:], recs.ap()[:])

            group_pT = {}

            def emit_oh(b):
                oh = ohpool.tile([128, 128 * K], bf16, name=f"oh{b}", tag="oh")
                nc.vector.tensor_tensor(
                    out=oh[:].rearrange("p (n c) -> p n c", c=K),
                    in0=iotar_t[:].rearrange("p (n c) -> p n c", c=K),
                    in1=lid_t[:, b * K:(b + 1) * K].unsqueeze(1).to_broadcast(
                        [128, 128, K]
                    ),
                    op=mybir.AluOpType.is_equal,
                )
                return oh

            def emit_matmuls(b, xt, oh):
                ps = psS.tile([128, 128], f32, name=f"ps{b}", tag="ps")
                half = (b % 2) * W128
                ohc = oh[:].rearrange("p (n c) -> p c n", c=K)
                for c in range(K):
                    nc.tensor.matmul(
                        ps[:],
                        lhsT=ohc[:, c, :],
                        rhs=xt[:, half + c * 128:half + (c + 1) * 128],
                        start=(c == 0),
                        stop=(c == K - 1),
                    )
                return ps

            def emit_epilogue(b, ps):
                # agg = sums * (1/max(deg,1)) straight from PSUM; rec is a
                # preloaded per-node constant, so the only cross-engine dep is
                # the PE accumulation stop.  agg/pT are bf16: the transpose
                # runs 1 cycle/row instead of fp32's 2.
                agg = epool.tile([128, 128], bf16, name=f"agg{b}", tag="agg")
                nc.scalar.activation(
                    agg[:], ps[:],
                    mybir.ActivationFunctionType.Copy, scale=rec_t[:, b:b + 1],
                )
                j = b % 4
                if j == 0:
                    group_pT["t"] = psT.tile([128, 512], bf16, name=f"pT{b}", tag="pT")
                pT = group_pT["t"]
                nc.tensor.transpose(pT[:, j * 128:(j + 1) * 128], agg[:], id_t[:])
                if j == 3 or b == B - 1:
                    g0 = (b // 4) * 4
                    gw = (b + 1 - g0) * 128
                    aggT = epool.tile([128, 512], bf16, name=f"aggT{b}", tag="aggT", bufs=2)
                    nc.scalar.copy(aggT[:, 0:gw], pT[:, 0:gw])
                    pO = psO.tile([128, 512], f32, name=f"pO{b}", tag="pO")
                    nc.tensor.matmul(
                        pO[:, 0:gw], lhsT=wt_t[:], rhs=aggT[:, 0:gw],
                        start=True, stop=True,
                    )
                    ot = epool.tile([128, 512], bf16, name=f"ot{b}", tag="ot", bufs=2)
                    nc.scalar.activation(
                        ot[:, 0:gw], pO[:, 0:gw],
                        mybir.ActivationFunctionType.Relu,
                        bias=bias_t[:, 0:1], scale=1.0,
                    )
                    # ACT HWDGE ring: output DMAs wait on the deep epilogue
                    # pipeline and would head-of-line block the input-pair
                    # DMAs on the SP ring.
                    nc.scalar.dma_start(out_ap[:, g0 * 128:(b + 1) * 128], ot[:, 0:gw])

            # Software-pipelined emission; per-engine queues are strict
            # in-order, so each stage's dependencies are blocks old by the
            # time its queue reaches the op:
            #   iter b:  DMA pair(b/2) | one-hot(b) | PE matmuls(b-1)
            #            | epilogue(b-3)
            pending_ps = {}
            xt_of = {}
            for b in range(B):
                if b % 2 == 0:
                    q = b // 2
                    xt = xpool.tile([128, 2 * W128], bf16, name=f"xt{q}", tag="xt")
                    nc.sync.dma_start(xt[:], xe_ap[q * 128:(q + 1) * 128, :])
                    xt_of[b] = xt
                    xt_of[b + 1] = xt
                oh = emit_oh(b)
                pending_ps[b] = emit_matmuls(b, xt_of.pop(b), oh)
                if b >= 3:
                    emit_epilogue(b - 3, pending_ps.pop(b - 3))
            for bb in sorted(pending_ps):
                emit_epilogue(bb, pending_ps.pop(bb))

    nc.compile()
    return nc


def _get_module(K):
    if K not in _module_cache:
        _module_cache[K] = _build_module(K)
    return _module_cache[K]


def _balance_nodes(deg):
    """Assign nodes to TOTAL_BLOCKS bins of <=128 nodes, equalizing per-bin
    edge load.  Returns (node_bin, node_loc, K)."""
    w = np.maximum(deg, 1).astype(np.int64)
    order = np.argsort(-w, kind="stable")
    S = TOTAL_BLOCKS * BLK
    idx = np.arange(S)
    r = idx // TOTAL_BLOCKS
    k = idx % TOTAL_BLOCKS
    bins = np.where(r % 2 == 0, k, TOTAL_BLOCKS - 1 - k)   # serpentine deal
    node_bin = np.empty(N_NODES, np.int64)
    node_loc = np.empty(N_NODES, np.int64)
    node_bin[order] = bins[:N_NODES]
    node_loc[order] = r[:N_NODES]
    loads = np.zeros(TOTAL_BLOCKS, np.int64)
    np.add.at(loads, node_bin, deg)

    cap = K_CHUNKS * BLK
    for _ in range(2000):
        hot = int(loads.argmax())
        if loads[hot] <= cap:
            break
        cold = int(loads.argmin())
        need = int(loads[hot] - cap)
        headroom = int(cap - loads[cold])
        hot_nodes = np.nonzero(node_bin == hot)[0]
        cold_nodes = np.nonzero(node_bin == cold)[0]
        diffs = deg[hot_nodes][:, None] - deg[cold_nodes][None, :]
        mask = (diffs >= 1) & (diffs <= headroom)
        if not mask.any():
            break
        cand = np.where(mask, np.abs(diffs - need), 1 << 30)
        ai, bi = np.unravel_index(np.argmin(cand), cand.shape)
        na, nb = hot_nodes[ai], cold_nodes[bi]
        node_bin[na], node_bin[nb] = node_bin[nb], node_bin[na]
        node_loc[na], node_loc[nb] = node_loc[nb], node_loc[na]
        d = int(deg[na] - deg[nb])
        loads[hot] -= d
        loads[cold] += d

    K = max(K_CHUNKS, int(np.ceil(loads.max() / BLK)))
    return node_bin, node_loc, K


def prepare_inputs(edge_data, dst, W, b):
    """Host-side sharding: route each edge to the core/block owning dst."""
    edge_data = np.asarray(edge_data, dtype=np.float32)
    dst = np.asarray(dst).astype(np.int64)
    W = np.asarray(W, dtype=np.float32)
    b = np.asarray(b, dtype=np.float32)
    E = dst.shape[0]

    deg = np.bincount(dst, minlength=N_NODES)[:N_NODES]
    node_bin, node_loc, K = _balance_nodes(deg)
    RB = K * 128
    TOT = TOTAL_BLOCKS * RB

    eblk = node_bin[dst]
    cnt = np.bincount(eblk, minlength=TOTAL_BLOCKS)
    starts = np.zeros(TOTAL_BLOCKS, np.int64)
    np.cumsum(cnt[:-1], out=starts[1:])
    eorder = np.argsort(eblk, kind="stable")
    rank = np.empty(E, np.int64)
    rank[eorder] = np.arange(E, dtype=np.int64) - np.repeat(starts, cnt)
    slot = eblk * RB + rank

    X = np.zeros((TOT, 128), BF16)
    X[slot] = edge_data.astype(BF16)
    # [block, chunk, part, 128] -> [pair, part, 2 * chunk * 128]: each SBUF
    # partition's data is one 8 KiB contiguous HBM run, ~1 MiB per DMA.
    Xb = X.reshape(TOTAL_BLOCKS, K, 128, 128).transpose(0, 2, 1, 3)
    Xb = Xb.reshape(N_CORES, BLOCKS_PER_CORE // 2, 2, 128, K * 128)
    xe_all = np.ascontiguousarray(Xb.transpose(0, 1, 3, 2, 4)).reshape(
        N_CORES, (BLOCKS_PER_CORE // 2) * 128, 2 * K * 128
    )

    lid_f = np.full(TOT, -1.0, np.float32)
    lid_f[slot] = node_loc[dst].astype(np.float32)
    lid_all = (
        lid_f.reshape(N_CORES, BLOCKS_PER_CORE, K, 128)
        .transpose(0, 3, 1, 2)
        .reshape(N_CORES, 128, BLOCKS_PER_CORE * K)
        .astype(BF16)
    )
    rec_full = np.ones(TOTAL_BLOCKS * BLK, np.float32)
    rec_full[node_bin * BLK + node_loc] = 1.0 / np.maximum(deg, 1)
    rec_all = (
        rec_full.reshape(N_CORES, BLOCKS_PER_CORE, BLK).transpose(0, 2, 1)
    )                                            # [core, 128, B]
    wt = np.ascontiguousarray(W.T).astype(BF16)
    bias = np.ascontiguousarray(b.reshape(128, 1))
    ident = np.eye(128, dtype=np.float32).astype(BF16)
    iotar = np.ascontiguousarray(
        np.broadcast_to(
            np.repeat(np.arange(128, dtype=np.float32), K), (128, 128 * K)
        )
    ).astype(BF16)

    in_maps = [
        {
            "xe": np.ascontiguousarray(xe_all[c]),
            "lid": np.ascontiguousarray(lid_all[c]),
            "recs": np.ascontiguousarray(rec_all[c]),
            "wt": wt,
            "bias": bias,
            "ident": ident,
            "iotar": iotar,
        }
        for c in range(N_CORES)
    ]
    pos = node_bin * BLK + node_loc          # output column of each node
    return K, in_maps, pos


def run(edge_data, dst, W, b, trace=False, tmpdir=None):
    from concourse.bass_utils import run_bass_kernel_spmd

    K, in_maps, pos = prepare_inputs(edge_data, dst, W, b)
    nc = _get_module(K)
    res = run_bass_kernel_spmd(
        nc, in_maps, core_ids=list(range(N_CORES)), trace=trace, tmpdir=tmpdir,
    )
    full = np.concatenate(
        [res.results[c]["out"] for c in range(N_CORES)], axis=1
    )                                         # [128, 100352] bf16
    out = full.T[pos].astype(np.float32)      # un-permute -> [100000, 128]
    return np.ascontiguousarray(out), res


def kernel(edge_data, dst, W, b):
    out, _ = run(edge_data, dst, W, b, trace=False)
    return out
